# revision 2
# baseline (speedup 1.0000x reference)
"""DEQ MLP — Trainium2 Bass kernel, v2 (Picard + folded weights + fp8).

Problem: z* = fixpoint of f(z) = relu(z@W1+b1)@W2+b2, z0 = x@W_in+b_in,
out = z*@W_out + b_out.  B=1024, D=1024.  Reference solves with Anderson
acceleration (m=6, 40 iters); but f is strongly contractive (~0.17/iter),
so plain Picard iteration reaches the bf16-precision fixed point in ~7
steps — no Anderson machinery needed.

Key restructurings (validated numerically against an fp64 oracle):
 1. h-space iteration with host-folded weights: substituting z = h@W2+b2
    into h' = relu(z@W1+b1) gives  h' = relu(h@(W2@W1) + (b2@W1+b1)) —
    ONE 1024x1024 matmul per iteration instead of two.  Input/output
    projections fold likewise: h0 = relu(x@(W_in@W1) + (b_in@W1+b1)),
    out = h*@(W2@W_out) + (b2@W_out+b_out).
 2. fp8(e4m3) DoubleRow matmuls (K=256/instr, 0.5 PE cycles/row) for the
    first N_FP8 iterations, with power-of-2 scales (W*2^11, act*2^5);
    one bf16 polish iteration pins the fixed point to the bf16 floor.
    Emulated end-to-end: rel err 2.8e-3 (gate 2e-2).
 3. Biases ride IN the matmul as an extra contraction pair (hi/lo fp8
    split on partition 0 x ones vector), so PSUM evictions are pure
    relu(psum * 2^-k) — big 2-chunk instructions alternating ACT/DVE.
 4. Pure data parallel: batch 1024 -> 128 rows/core on 8 cores; weights
    replicated; no collectives.  All layouts feature-major (T-layout),
    zero transposes on device.
"""

import os
import sys

for _p in ("/opt/trn_rl_repo", "/root/.axon_site/_ro/trn_rl_repo"):
    if os.path.isdir(_p) and _p not in sys.path:
        sys.path.insert(0, _p)

import numpy as np
import ml_dtypes

import concourse.bass as bass
import concourse.mybir as mybir
from concourse.tile import TileContext

BF16 = mybir.dt.bfloat16
FP8 = mybir.dt.float8e4
F32 = mybir.dt.float32
AL = mybir.AluOpType
AF = mybir.ActivationFunctionType
DR = mybir.MatmulPerfMode.DoubleRow

P = 128
D = 1024           # hidden width (h space)
DIN = 512
DOUT = 512
NCD = D // P       # 8
NCI = DIN // P     # 4
NCO = DOUT // P    # 4
N_CORES = 8
B = 1024 // N_CORES  # 128 batch rows per core

# power-of-2 scales for fp8: weights *2^11, activations *2^5, psum *2^16
SWL, SZL = 11, 5
SPL = SWL + SZL

N_FP8 = 6          # fp8 Picard iterations (incl. none of the in-proj)
N_BF16 = 1         # bf16 polish iterations

bf16 = ml_dtypes.bfloat16
fp8 = ml_dtypes.float8_e4m3


def _emit(nc: bass.Bass, tc, ctx, n8: int, nb: int):
    # ---------------- DRAM I/O ----------------
    d_x8 = nc.declare_dram_parameter("x8", [P, NCI * B], FP8, isOutput=False)
    d_ones8 = nc.declare_dram_parameter("ones8", [P, 2 * B], FP8, isOutput=False)
    d_onesb = nc.declare_dram_parameter("onesb", [P, B], BF16, isOutput=False)
    d_win18 = nc.declare_dram_parameter("win18", [P, 2 * 2 * NCD * P], FP8, isOutput=False)
    d_cin8 = nc.declare_dram_parameter("cin8", [P, 2 * NCD * P], FP8, isOutput=False)
    d_w218 = nc.declare_dram_parameter("w218", [P, 4 * 2 * NCD * P], FP8, isOutput=False)
    d_c8 = nc.declare_dram_parameter("c8", [P, 2 * NCD * P], FP8, isOutput=False)
    d_w21b = nc.declare_dram_parameter("w21b", [P, NCD * NCD * P], BF16, isOutput=False)
    d_cb = nc.declare_dram_parameter("cb", [P, NCD * P], BF16, isOutput=False)
    d_w2outb = nc.declare_dram_parameter("w2outb", [P, NCD * NCO * P], BF16, isOutput=False)
    d_coutb = nc.declare_dram_parameter("coutb", [P, NCO * P], BF16, isOutput=False)
    d_out = nc.declare_dram_parameter("out", [P, NCO * B], F32, isOutput=True)

    consts = ctx.enter_context(tc.tile_pool(name="consts", bufs=1))
    state = ctx.enter_context(tc.tile_pool(name="state", bufs=1))
    h8p = ctx.enter_context(tc.tile_pool(name="h8p", bufs=2))
    hbp = ctx.enter_context(tc.tile_pool(name="hbp", bufs=2))
    pp = ctx.enter_context(tc.tile_pool(name="pp", bufs=2, space="PSUM"))
    op = ctx.enter_context(tc.tile_pool(name="op", bufs=1, space="PSUM"))

    # ---------------- constants into SBUF ----------------
    x8 = consts.tile([P, NCI, B], FP8)
    ones8 = consts.tile([P, 2, B], FP8)
    onesb = consts.tile([P, B], BF16)
    Win18 = consts.tile([P, 2, 2, NCD, P], FP8)
    Cin8 = consts.tile([P, 2, NCD, P], FP8)
    W218 = consts.tile([P, 4, 2, NCD, P], FP8)
    C8 = consts.tile([P, 2, NCD, P], FP8)
    W21b = consts.tile([P, NCD, NCD, P], BF16)
    Cb = consts.tile([P, NCD, P], BF16)
    W2outb = consts.tile([P, NCD, NCO, P], BF16)
    Coutb = consts.tile([P, NCO, P], BF16)
    nc.sync.dma_start(out=x8[:, :, :], in_=d_x8[:, :])
    nc.sync.dma_start(out=ones8[:, :, :], in_=d_ones8[:, :])
    nc.sync.dma_start(out=Win18[:, :, :, :, :], in_=d_win18[:, :])
    nc.sync.dma_start(out=Cin8[:, :, :, :], in_=d_cin8[:, :])
    nc.sync.dma_start(out=W218[:, :, :, :, :], in_=d_w218[:, :])
    nc.sync.dma_start(out=C8[:, :, :, :], in_=d_c8[:, :])
    nc.sync.dma_start(out=onesb[:, :], in_=d_onesb[:, :])
    nc.sync.dma_start(out=Cb[:, :, :], in_=d_cb[:, :])
    nc.sync.dma_start(out=W21b[:, :, :, :], in_=d_w21b[:, :])
    nc.sync.dma_start(out=W2outb[:, :, :, :], in_=d_w2outb[:, :])
    nc.sync.dma_start(out=Coutb[:, :, :], in_=d_coutb[:, :])

    def evict4(pt, out_t, scale, relu, nchunks=NCD):
        """PSUM -> SBUF, 2 chunks per instruction, alternating ACT/DVE."""
        for i in range(0, nchunks, 2):
            sl = slice(i, i + 2)
            if (i // 2) % 2 == 0:
                nc.scalar.activation(
                    out_t[:, sl, :], pt[:, sl, :],
                    AF.Relu if relu else AF.Copy, bias=0.0, scale=scale,
                )
            elif relu:
                nc.vector.tensor_scalar(
                    out=out_t[:, sl, :], in0=pt[:, sl, :],
                    scalar1=scale, scalar2=0.0, op0=AL.mult, op1=AL.max,
                )
            else:
                nc.vector.tensor_scalar(
                    out=out_t[:, sl, :], in0=pt[:, sl, :],
                    scalar1=scale, scalar2=None, op0=AL.mult,
                )

    def fp8_layer(Wt, biast, rhs, npairs, out_t, out_scale):
        pt = pp.tile([P, NCD, B], F32)
        for n in range(NCD):
            nc.tensor.matmul(pt[:, n, :], lhsT=biast[:, :, n, :],
                             rhs=ones8[:, :, :], start=True, stop=False,
                             perf_mode=DR)
            for cp in range(npairs):
                nc.tensor.matmul(pt[:, n, :], lhsT=Wt[:, cp, :, n, :],
                                 rhs=rhs[:, 2 * cp : 2 * cp + 2, :],
                                 start=False, stop=(cp == npairs - 1),
                                 perf_mode=DR)
        evict4(pt, out_t, out_scale, relu=True)

    def bf16_layer(rhs, out_t):
        pt = pp.tile([P, NCD, B], F32)
        for n in range(NCD):
            nc.tensor.matmul(pt[:, n, :], lhsT=Cb[:, n, :], rhs=onesb[:, :],
                             start=True, stop=False)
            for c in range(NCD):
                nc.tensor.matmul(pt[:, n, :], lhsT=W21b[:, c, n, :],
                                 rhs=rhs[:, c, :], start=False,
                                 stop=(c == NCD - 1))
        evict4(pt, out_t, 1.0, relu=True)

    # ---------------- program ----------------
    # in-proj: h = relu(x @ W_in1 + c_in), fp8, K=512 (2 pairs)
    h8 = h8p.tile([P, NCD, B], FP8)
    fp8_layer(Win18, Cin8, x8, 2, h8, 2.0 ** (SZL - SPL))

    # fp8 Picard iterations (last one evicts to bf16 for the polish/out)
    for i in range(n8):
        last = i == n8 - 1
        if last:
            out_t = hbp.tile([P, NCD, B], BF16)
            out_scale = 2.0 ** (-SPL)
        else:
            out_t = h8p.tile([P, NCD, B], FP8)
            out_scale = 2.0 ** (SZL - SPL)
        pt = pp.tile([P, NCD, B], F32)
        for n in range(NCD):
            nc.tensor.matmul(pt[:, n, :], lhsT=C8[:, :, n, :],
                             rhs=ones8[:, :, :], start=True, stop=False,
                             perf_mode=DR)
            for cp in range(4):
                nc.tensor.matmul(pt[:, n, :], lhsT=W218[:, cp, :, n, :],
                                 rhs=h8[:, 2 * cp : 2 * cp + 2, :],
                                 start=False, stop=(cp == 3), perf_mode=DR)
        evict4(pt, out_t, out_scale, relu=True)
        if last:
            hb = out_t
        else:
            h8 = out_t

    # bf16 polish iterations
    for j in range(nb):
        nxt = hbp.tile([P, NCD, B], BF16)
        bf16_layer(hb, nxt)
        hb = nxt

    # out-proj: out = h @ W2out + c_out  (bf16 weights, fp32 out)
    ot = op.tile([P, NCO, B], F32)
    for o in range(NCO):
        nc.tensor.matmul(ot[:, o, :], lhsT=Coutb[:, o, :], rhs=onesb[:, :],
                         start=True, stop=False)
        for c in range(NCD):
            nc.tensor.matmul(ot[:, o, :], lhsT=W2outb[:, c, o, :],
                             rhs=hb[:, c, :], start=False, stop=(c == NCD - 1))
    outT = state.tile([P, NCO, B], F32)
    evict4(ot, outT, 1.0, relu=False, nchunks=NCO)
    nc.sync.dma_start(out=d_out[:, :], in_=outT[:, :, :])


def build_program(n8: int = N_FP8, nb: int = N_BF16) -> bass.Bass:
    from contextlib import ExitStack

    from concourse import bacc

    nc = bacc.Bacc(trn_type="TRN2", target_bir_lowering=False)
    with ExitStack() as ctx:
        tc = ctx.enter_context(TileContext(nc))
        _emit(nc, tc, ctx, n8, nb)
    nc.compile()
    return nc


def _fold_weights(inputs):
    """Host-side weight folding in fp64 (cheap: ~3.5 GFLOP once per call)."""
    f64 = np.float64
    W_in, b_in = inputs["W_in"].astype(f64), inputs["b_in"].astype(f64)
    W1, b1 = inputs["W1"].astype(f64), inputs["b1"].astype(f64)
    W2, b2 = inputs["W2"].astype(f64), inputs["b2"].astype(f64)
    W_out, b_out = inputs["W_out"].astype(f64), inputs["b_out"].astype(f64)
    return {
        "W_in1": W_in @ W1, "c_in": b_in @ W1 + b1,
        "W21": W2 @ W1, "c": b2 @ W1 + b1,
        "W2out": W2 @ W_out, "c_out": b2 @ W_out + b_out,
    }


def _pack_w8(W, sw):
    """[K, N] -> [128, K//256, 2, N//128, 128] fp8 (DoubleRow pairs)."""
    K, N = W.shape
    t = (W * sw).astype(fp8)
    t = t.reshape(K // 256, 2, P, N // P, P).transpose(2, 0, 1, 3, 4)
    return np.ascontiguousarray(t.reshape(P, -1))


def _pack_wb(W):
    """[K, N] -> [128, K//128, N//128, 128] bf16."""
    K, N = W.shape
    t = W.astype(bf16).reshape(K // P, P, N // P, P).transpose(1, 0, 2, 3)
    return np.ascontiguousarray(t.reshape(P, -1))


def _pack_bias8(c):
    """[N] -> [128, 2, N//128, 128] fp8: partition 0 = (hi, lo) rows.
    total contribution = hi*2^7 + lo*2^4 = c*2^16 (+O(0.4%) of the hi
    rounding) when paired with the (128, 16) ones vector."""
    hi64 = c * 2.0 ** (SPL - 7)
    hi = hi64.astype(fp8)
    lo = ((hi64 - hi.astype(np.float64)) * 2.0 ** 3).astype(fp8)
    arr = np.zeros((P, 2, c.shape[0] // P, P), fp8)
    arr[0, 0] = hi.reshape(-1, P)
    arr[0, 1] = lo.reshape(-1, P)
    return np.ascontiguousarray(arr.reshape(P, -1))


def _pack_biasb(c):
    arr = np.zeros((P, c.shape[0] // P, P), bf16)
    arr[0] = c.astype(bf16).reshape(-1, P)
    return np.ascontiguousarray(arr.reshape(P, -1))


def _prep_inputs(inputs):
    F = _fold_weights(inputs)
    ones8 = np.zeros((P, 2, B), fp8)
    ones8[0, 0, :] = 2.0 ** 7
    ones8[0, 1, :] = 2.0 ** 4
    onesb = np.zeros((P, B), bf16)
    onesb[0, :] = 1.0
    sw = 2.0 ** SWL
    shared = {
        "ones8": np.ascontiguousarray(ones8.reshape(P, -1)),
        "onesb": onesb,
        "win18": _pack_w8(F["W_in1"], sw),
        "cin8": _pack_bias8(F["c_in"]),
        "w218": _pack_w8(F["W21"], sw),
        "c8": _pack_bias8(F["c"]),
        "w21b": _pack_wb(F["W21"]),
        "cb": _pack_biasb(F["c"]),
        "w2outb": _pack_wb(F["W2out"]),
        "coutb": _pack_biasb(F["c_out"]),
    }
    x = inputs["x"]
    in_maps = []
    for cidx in range(N_CORES):
        xs = x[cidx * B : (cidx + 1) * B].astype(np.float64)   # [128, 512]
        x8t = (xs.T * 2.0 ** SZL).astype(fp8)                  # [512, 128]
        x8t = x8t.reshape(NCI, P, B).transpose(1, 0, 2)        # [128, 4, 128]
        im = {"x8": np.ascontiguousarray(x8t.reshape(P, -1))}
        im.update(shared)
        in_maps.append(im)
    return in_maps


_CACHE = {}


def run_on_hw(inputs, n8: int = N_FP8, nb: int = N_BF16, trace: bool = False):
    """Returns (output [1024, 512] fp32, BassKernelResults)."""
    from concourse.bass_utils import run_bass_kernel_spmd

    key = (n8, nb)
    if key not in _CACHE:
        _CACHE[key] = build_program(n8, nb)
    nc = _CACHE[key]
    in_maps = _prep_inputs(inputs)
    res = run_bass_kernel_spmd(nc, in_maps, list(range(N_CORES)), trace=trace)
    outs = []
    for i in range(N_CORES):
        oT = np.asarray(res.results[i]["out"], dtype=np.float32)  # [128, 4*128]
        oT = oT.reshape(P, NCO, B).transpose(2, 1, 0).reshape(B, DOUT)
        outs.append(oT)
    return np.concatenate(outs, axis=0), res


def bench_on_hw(inputs, n8: int = N_FP8, nb: int = N_BF16, reps: int = 32):
    """Per-execution device time via pipelined repeated execution."""
    import time

    import jax
    from jax.sharding import Mesh, PartitionSpec
    from jax.experimental.shard_map import shard_map

    from concourse import bass2jax, mybir as mb

    key = (n8, nb)
    if key not in _CACHE:
        _CACHE[key] = build_program(n8, nb)
    nc = _CACHE[key]
    bass2jax.install_neuronx_cc_hook()

    partition_name = nc.partition_id_tensor.name if nc.partition_id_tensor else None
    in_names, out_names, out_avals, zero_outs = [], [], [], []
    for alloc in nc.m.functions[0].allocations:
        if not isinstance(alloc, mb.MemoryLocationSet):
            continue
        name = alloc.memorylocations[0].name
        if alloc.kind == "ExternalInput":
            if name != partition_name:
                in_names.append(name)
        elif alloc.kind == "ExternalOutput":
            out_names.append(name)
            shape = tuple(alloc.tensor_shape)
            dtype = mb.dt.np(alloc.dtype)
            out_avals.append(jax.core.ShapedArray(shape, dtype))
            zero_outs.append(np.zeros(shape, dtype))
    n_params = len(in_names)
    in_names_all = in_names + out_names
    if partition_name is not None:
        in_names_all.append(partition_name)

    def _body(*args):
        operands = list(args)
        if partition_name is not None:
            operands.append(bass2jax.partition_id_tensor())
        outs = bass2jax._bass_exec_p.bind(
            *operands,
            out_avals=tuple(out_avals),
            in_names=tuple(in_names_all),
            out_names=tuple(out_names),
            lowering_input_output_aliases=(),
            sim_require_finite=True,
            sim_require_nnan=True,
            nc=nc,
        )
        return tuple(outs)

    in_maps = _prep_inputs(inputs)
    devices = jax.devices()[:N_CORES]
    mesh = Mesh(np.asarray(devices), ("core",))
    in_specs = (PartitionSpec("core"),) * (n_params + len(out_names))
    out_specs = (PartitionSpec("core"),) * len(out_names)
    sharded = jax.jit(
        shard_map(_body, mesh=mesh, in_specs=in_specs, out_specs=out_specs,
                  check_rep=False),
        keep_unused=True,
    )
    concat_in = [
        np.concatenate([np.asarray(in_maps[c][nm]) for c in range(N_CORES)], axis=0)
        for nm in in_names
    ]
    concat_zeros = [
        np.zeros((N_CORES * z.shape[0], *z.shape[1:]), z.dtype) for z in zero_outs
    ]
    args = [jax.device_put(a) for a in concat_in + concat_zeros]
    out = sharded(*args)
    jax.block_until_ready(out)
    best = float("inf")
    for _ in range(3):
        t0 = time.perf_counter()
        outs = [sharded(*args) for _ in range(reps)]
        jax.block_until_ready(outs)
        dt = (time.perf_counter() - t0) / reps
        best = min(best, dt)
    out_np = np.asarray(out[0], dtype=np.float32)
    return best, out_np


def kernel(**inputs) -> np.ndarray:
    out, _ = run_on_hw(inputs)
    return out


if __name__ == "__main__":
    nc = build_program()
    print("built ok")
